# revision 1
# baseline (speedup 1.0000x reference)
"""ChromosomeEmbedding kernel for 8x Trainium2 NeuronCores.

Computes out[b, j, d] = ce[chr[b]-1, d] for b in [0,512), j in [0,2001),
d in [0,128). Data-parallel: the batch is sharded 64 samples/core across
8 cores; the tiny 24x128 table ce is replicated to every core.

Per-core device program (identical SPMD program on all cores):
  1. One DMA loads a packed prelude tensor: chr broadcast to [32, 128],
     an iota column (1..32), and the table zero-padded to 32 rows.
  2. One-hot gather on the tensor engine: onehotT[k, p] = (chr[p%64] == k+1)
     via a single is_equal tensor_scalar, then rows = onehotT.T @ ce as a
     fp32 matmul (exact -- exactly one 1.0 per one-hot column). Partition p
     of the PSUM result holds the embedding row of sample p % 64.
  3. Six doubling copies on the vector engine replicate each partition's
     row 64x along the free dim -> rep[128, 64, 128] (32 KB/partition).
  4. ~34 large DMAs stream the [64, 2001, 128] output shard (65.5 MB),
     split between the two HWDGE rings: sync walks bins [0, 1001) from
     partitions 0:64, scalar walks [1001, 2001) from partitions 64:128.
     The SDMA engines round-robin between the two queue rings at packet
     granularity, so each ring opens with a 32-bin pass that depends only
     on the w=32 doubling copy -- both queues enter the engine rotation
     ~2 us earlier and the stream sustains ~340-360 GB/s instead of
     ~300 GB/s single-queue phases at the edges.

Measured on trn2: ~213 us HW exec for the 524 MB full output
(~345 GB/s/core steady-state HBM write rate), bit-exact vs reference.
"""

import functools

import numpy as np

from concourse import bacc, mybir, tile
from concourse.bass_utils import run_bass_kernel_spmd

N_CORES = 8
BS = 512
BPC = BS // N_CORES  # 64 samples per core
NBIN = 2001
DIM = 128
N_CHR = 24
KPAD = 32  # contraction dim: 24 table rows zero-padded to 32
REP = 64  # replicated copies of each row held in SBUF
PRE_W = 132 + DIM  # prelude row: 128 chr | iota | 3 pad | 128 table
SPLIT = 1001  # bins walked by the sync ring; scalar ring takes the rest
OPENERS = True  # small w=32-dependent opener pass per ring
F32 = mybir.dt.float32


@functools.lru_cache(maxsize=1)
def build_nc():
    nc = bacc.Bacc("TRN2", target_bir_lowering=False)

    pre_h = nc.declare_dram_parameter("pre", [KPAD, PRE_W], F32, isOutput=False)
    out_h = nc.declare_dram_parameter("out", [BPC, NBIN, DIM], F32, isOutput=True)

    with tile.TileContext(nc) as tc:
        with (
            tc.tile_pool(name="pool", bufs=1) as pool,
            tc.tile_pool(name="psum", bufs=1, space="PSUM") as psum,
        ):
            pre = pool.tile([KPAD, PRE_W], F32, tag="pre")
            oh = pool.tile([KPAD, 128], F32, tag="oh")
            rows_ps = psum.tile([128, DIM], F32, tag="rows")
            rep = pool.tile([128, REP, DIM], F32, tag="rep")

            # One-hot gather on the PE: onehotT[k, p] = (chr[p % 64] == k+1),
            # rows = onehotT.T @ ce (exact: one 1.0 per column). chr arrives
            # broadcast to [32, 128] (host-side replication, same as
            # replicating ce across cores).
            nc.sync.dma_start(out=pre[:, :], in_=pre_h[:, :])
            nc.vector.tensor_scalar(
                out=oh[:, :],
                in0=pre[:, 0:128],
                scalar1=pre[:, 128:129],
                scalar2=None,
                op0=mybir.AluOpType.is_equal,
            )
            nc.tensor.matmul(
                rows_ps[:, :],
                oh[:, :],
                pre[:, 132 : 132 + DIM],
                start=True,
                stop=True,
            )
            nc.vector.tensor_copy(out=rep[:, 0:1, :], in_=rows_ps[:, :])

            # Replicate each partition's row REP times along the free dim.
            w = 1
            while w < REP:
                nc.vector.tensor_copy(out=rep[:, w : 2 * w, :], in_=rep[:, 0:w, :])
                w *= 2

            # Stream the output. The low partition half is pinned to the
            # sync HWDGE ring and the high half to the scalar ring; the sync
            # ring enters the engine rotation ~8 us earlier (it also carries
            # the prelude DMA), so it gets more bins so both rings drain dry
            # at the same time.
            half = SPLIT
            OPEN = 32 if OPENERS else 0
            if OPEN:
                # 32-bin openers depend only on the w=32 copy, putting both
                # queues into the SDMA engines' rotation ~2 us earlier.
                nc.sync.dma_start(
                    out=out_h[:, 0:OPEN, :], in_=rep[0:BPC, 0:OPEN, :]
                )
                nc.scalar.dma_start(
                    out=out_h[:, half : half + OPEN, :], in_=rep[BPC:128, 0:OPEN, :]
                )
            for t in range((half - OPEN) // REP):
                nc.sync.dma_start(
                    out=out_h[:, OPEN + t * REP : OPEN + (t + 1) * REP, :],
                    in_=rep[0:BPC, :, :],
                )
            r0 = (half - OPEN) % REP
            if r0:
                nc.sync.dma_start(
                    out=out_h[:, half - r0 : half, :], in_=rep[0:BPC, 0:r0, :]
                )
            for t in range((NBIN - half - OPEN) // REP):
                nc.scalar.dma_start(
                    out=out_h[
                        :, half + OPEN + t * REP : half + OPEN + (t + 1) * REP, :
                    ],
                    in_=rep[BPC:128, :, :],
                )
            r1 = (NBIN - half - OPEN) % REP
            if r1:
                nc.scalar.dma_start(
                    out=out_h[:, NBIN - r1 : NBIN, :], in_=rep[BPC:128, 0:r1, :]
                )

    nc.compile()
    return nc


def make_in_maps(chr_full: np.ndarray, ce: np.ndarray):
    chr_f32 = chr_full.astype(np.float32)
    ce_pad = np.zeros((KPAD, DIM), np.float32)
    ce_pad[:N_CHR] = ce.astype(np.float32)
    maps = []
    for c in range(N_CORES):
        shard = chr_f32[c * BPC : (c + 1) * BPC]
        pre = np.zeros((KPAD, PRE_W), np.float32)
        pre[:, 0:128] = np.tile(shard, (KPAD, 2))  # chr broadcast
        pre[:, 128] = np.arange(1, KPAD + 1)  # iota
        pre[:, 132 : 132 + DIM] = ce_pad
        maps.append({"pre": np.ascontiguousarray(pre)})
    return maps


def kernel(tensor=None, chr=None, ce=None, **_unused):
    chr_np = np.asarray(chr)
    ce_np = np.asarray(ce)
    nc = build_nc()
    res = run_bass_kernel_spmd(
        nc, make_in_maps(chr_np, ce_np), core_ids=list(range(N_CORES))
    )
    out = np.concatenate([r["out"] for r in res.results], axis=0)
    return out.astype(np.float32)



# revision 2
# speedup vs baseline: 1.0453x; 1.0453x over previous
"""ChromosomeEmbedding kernel for 8x Trainium2 NeuronCores.

Computes out[b, j, d] = ce[chr[b]-1, d] for b in [0,512), j in [0,2001),
d in [0,128). Data-parallel: the batch is sharded 64 samples/core across
8 cores; the tiny 24x128 table ce is replicated to every core.

Per-core device program (identical SPMD program on all cores):
  1. One DMA loads a packed prelude tensor: chr broadcast to [32, 128],
     an iota column (1..32), and the table zero-padded to 32 rows.
  2. One-hot gather on the tensor engine: onehotT[k, p] = (chr[p%64] == k+1)
     via a single is_equal tensor_scalar, then rows = onehotT.T @ ce as a
     fp32 matmul (exact -- exactly one 1.0 per one-hot column). Partition p
     of the PSUM result holds the embedding row of sample p % 64.
  3. Five doubling copies on the vector engine replicate each partition's
     row 32x along the free dim -> rep[128, 32, 128] (16 KB/partition).
  4. The [64, 2001, 128] output shard (65.5 MB) streams out on the two
     HWDGE rings: sync walks bins [0, 1000) from partitions 0:64, scalar
     walks [1000, 2001) from partitions 64:128. Each ring issues small
     "opener" DMAs pipelined against the doubling chain (8 bins after
     the w=8 copy, 16 after w=16, 32+16 after w=32), then ONE giant
     InstDMACopy covering its remaining 928 bins via a stride-0
     broadcast source AP (rep[...,0:32,:] re-read 29x). A single giant
     instruction per ring avoids the per-instruction completion-
     semaphore stall (an HBM write-receipt round trip per ring per
     2 MB in the many-instruction version) that capped the baseline at
     ~340 GB/s aggregate.
"""

import functools

import numpy as np

from concourse import bacc, mybir, tile
from concourse.bass_utils import run_bass_kernel_spmd

N_CORES = 8
BS = 512
BPC = BS // N_CORES  # 64 samples per core
NBIN = 2001
DIM = 128
N_CHR = 24
KPAD = 32  # contraction dim: 24 table rows zero-padded to 32
W = 32  # replicated copies of each row held in SBUF (16 KB/partition)
PRE_W = 132 + DIM  # prelude row: 128 chr | iota | 3 pad | 128 table
SYNC_BINS = 1000  # bins walked by the sync ring; scalar takes the rest
F32 = mybir.dt.float32


def _emit_ring(nc, engine, out_h, rep, plo, phi, b0, b1):
    """Emit one ring's DMA program: pipelined openers then one giant
    broadcast-source InstDMACopy. Partitions [plo, phi) of rep serve
    output bins [b0, b1)."""
    total = b1 - b0
    # Openers sized to land as each doubling copy completes.
    openers = []
    done = 0
    for wav in (8, 16, 32):
        n = min(wav, total - done)
        if n <= 0:
            break
        openers.append(n)
        done += n
    # Tail chunk to make the giant's bin count a multiple of W.
    rem = (total - done) % W
    if rem:
        openers.append(rem)
        done += rem
    giant = total - done  # multiple of W
    pos = b0
    for n in openers:
        engine.dma_start(
            out=out_h[:, pos : pos + n, :], in_=rep[plo:phi, 0:n, :]
        )
        pos += n
    if giant:
        r = giant // W
        src = rep[plo:phi, :, :].unsqueeze(1).broadcast_to(
            (phi - plo, r, W, DIM)
        )
        engine.dma_start(out=out_h[:, pos : pos + giant, :], in_=src)


@functools.lru_cache(maxsize=1)
def build_nc():
    nc = bacc.Bacc("TRN2", target_bir_lowering=False)

    pre_h = nc.declare_dram_parameter("pre", [KPAD, PRE_W], F32, isOutput=False)
    out_h = nc.declare_dram_parameter("out", [BPC, NBIN, DIM], F32, isOutput=True)

    with tile.TileContext(nc) as tc:
        with (
            tc.tile_pool(name="pool", bufs=1) as pool,
            tc.tile_pool(name="psum", bufs=1, space="PSUM") as psum,
        ):
            pre = pool.tile([KPAD, PRE_W], F32, tag="pre")
            oh = pool.tile([KPAD, 128], F32, tag="oh")
            rows_ps = psum.tile([128, DIM], F32, tag="rows")
            rep = pool.tile([128, W, DIM], F32, tag="rep")

            # One-hot gather on the PE: onehotT[k, p] = (chr[p % 64] == k+1),
            # rows = onehotT.T @ ce (exact: one 1.0 per column). chr arrives
            # broadcast to [32, 128] (host-side replication, same as
            # replicating ce across cores).
            nc.sync.dma_start(out=pre[:, :], in_=pre_h[:, :])
            nc.vector.tensor_scalar(
                out=oh[:, :],
                in0=pre[:, 0:128],
                scalar1=pre[:, 128:129],
                scalar2=None,
                op0=mybir.AluOpType.is_equal,
            )
            nc.tensor.matmul(
                rows_ps[:, :],
                oh[:, :],
                pre[:, 132 : 132 + DIM],
                start=True,
                stop=True,
            )
            nc.vector.tensor_copy(out=rep[:, 0:1, :], in_=rows_ps[:, :])

            # Replicate each partition's row W times along the free dim.
            w = 1
            while w < W:
                nc.vector.tensor_copy(out=rep[:, w : 2 * w, :], in_=rep[:, 0:w, :])
                w *= 2

            # Stream the output. The low partition half is pinned to the
            # sync HWDGE ring and the high half to the scalar ring so each
            # ring's source spans one SBUF port group; the openers enter
            # the SDMA rotation while the doubling chain is still running.
            _emit_ring(nc, nc.sync, out_h, rep, 0, BPC, 0, SYNC_BINS)
            _emit_ring(nc, nc.scalar, out_h, rep, BPC, 128, SYNC_BINS, NBIN)

    nc.compile()
    return nc


def make_in_maps(chr_full: np.ndarray, ce: np.ndarray):
    chr_f32 = chr_full.astype(np.float32)
    ce_pad = np.zeros((KPAD, DIM), np.float32)
    ce_pad[:N_CHR] = ce.astype(np.float32)
    maps = []
    for c in range(N_CORES):
        shard = chr_f32[c * BPC : (c + 1) * BPC]
        pre = np.zeros((KPAD, PRE_W), np.float32)
        pre[:, 0:128] = np.tile(shard, (KPAD, 2))  # chr broadcast
        pre[:, 128] = np.arange(1, KPAD + 1)  # iota
        pre[:, 132 : 132 + DIM] = ce_pad
        maps.append({"pre": np.ascontiguousarray(pre)})
    return maps


def kernel(tensor=None, chr=None, ce=None, **_unused):
    chr_np = np.asarray(chr)
    ce_np = np.asarray(ce)
    nc = build_nc()
    res = run_bass_kernel_spmd(
        nc, make_in_maps(chr_np, ce_np), core_ids=list(range(N_CORES))
    )
    out = np.concatenate([r["out"] for r in res.results], axis=0)
    return out.astype(np.float32)


# revision 5
# speedup vs baseline: 1.1561x; 1.1060x over previous
"""ChromosomeEmbedding kernel for 8x Trainium2 NeuronCores.

Computes out[b, j, d] = ce[chr[b]-1, d] for b in [0,512), j in [0,2001),
d in [0,128). Data-parallel: the batch is sharded 64 samples/core across
8 cores; the tiny 24x128 table ce is replicated to every core.

Per-core device program (identical SPMD program on all cores):
  1. One DMA loads a packed prelude tensor: chr broadcast to [32, 128],
     an iota column (1..32), and the table zero-padded to 32 rows.
  2. One-hot gather on the tensor engine: onehotT[k, p] = (chr[p%64] == k+1)
     via a single is_equal tensor_scalar, then rows = onehotT.T @ ce as a
     fp32 matmul (exact -- exactly one 1.0 per one-hot column). Partition p
     of the PSUM result holds the embedding row of sample p % 64.
  3. Seven doubling copies on the vector engine replicate each partition's
     row 96x along the free dim -> rep[128, 96, 128] (48 KB/partition).
  4. The [64, 2001, 128] output shard (65.5 MB) streams out on the two
     HWDGE rings: sync walks bins [0, 1000) from partitions 0:64, scalar
     walks [1000, 2001) from partitions 64:128. Each ring issues small
     "opener" DMAs pipelined against the doubling chain (8 bins after
     the w=8 copy, 16 after w=16, 32+16 after w=32), then ONE giant
     InstDMACopy covering its remaining 928 bins via a stride-0
     broadcast source AP (rep[...,0:32,:] re-read 29x). A single giant
     instruction per ring avoids the per-instruction completion-
     semaphore stall (an HBM write-receipt round trip per ring per
     2 MB in the many-instruction version) that capped the baseline at
     ~340 GB/s aggregate.
"""

import functools

import numpy as np

from concourse import bacc, mybir, tile
from concourse.bass_utils import run_bass_kernel_spmd

N_CORES = 8
BS = 512
BPC = BS // N_CORES  # 64 samples per core
NBIN = 2001
DIM = 128
N_CHR = 24
KPAD = 32  # contraction dim: 24 table rows zero-padded to 32
W = 96  # replicated copies of each row held in SBUF (48 KB/partition)
PRE_W = 132 + DIM  # prelude row: 128 chr | iota | 3 pad | 128 table
SYNC_BINS = 1000  # bins walked by the sync ring; scalar takes the rest
F32 = mybir.dt.float32


def _emit_ring(nc, engine, out_h, rep, plo, phi, b0, b1):
    """Emit one ring's DMA program: pipelined openers then one giant
    broadcast-source InstDMACopy. Partitions [plo, phi) of rep serve
    output bins [b0, b1). The opener ladder (8/16/32/64 bins) lands as
    each doubling copy completes, keeping the SDMA engines fed while
    the replication chain is still running; the 48 KB descriptors of
    the giant keep HWDGE descriptor generation comfortably ahead of
    the 16 engines' consumption (small descriptors starve the last
    engine of each generation round, which then drags out the tail)."""
    total = b1 - b0
    # Openers sized to land as each doubling copy completes.
    openers = []
    done = 0
    for wav in (8, 16, 32):
        n = min(wav, total - done)
        if n <= 0:
            break
        openers.append(n)
        done += n
    # Tail chunk to make the giant's bin count a multiple of W.
    rem = (total - done - 64) % W
    if rem:
        openers.append(rem)
        done += rem
    if total - done >= 64:
        openers.append(64)  # lands right as the w=64 copy completes
        done += 64
    giant = total - done  # multiple of W
    pos = b0
    for n in openers:
        engine.dma_start(
            out=out_h[:, pos : pos + n, :], in_=rep[plo:phi, 0:n, :]
        )
        pos += n
    if giant:
        r = giant // W
        src = rep[plo:phi, :, :].unsqueeze(1).broadcast_to(
            (phi - plo, r, W, DIM)
        )
        engine.dma_start(out=out_h[:, pos : pos + giant, :], in_=src)


@functools.lru_cache(maxsize=1)
def build_nc():
    nc = bacc.Bacc("TRN2", target_bir_lowering=False)

    pre_h = nc.declare_dram_parameter("pre", [KPAD, PRE_W], F32, isOutput=False)
    out_h = nc.declare_dram_parameter("out", [BPC, NBIN, DIM], F32, isOutput=True)

    with tile.TileContext(nc) as tc:
        with (
            tc.tile_pool(name="pool", bufs=1) as pool,
            tc.tile_pool(name="psum", bufs=1, space="PSUM") as psum,
        ):
            pre = pool.tile([KPAD, PRE_W], F32, tag="pre")
            oh = pool.tile([KPAD, 128], F32, tag="oh")
            rows_ps = psum.tile([128, DIM], F32, tag="rows")
            rep = pool.tile([128, W, DIM], F32, tag="rep")

            # One-hot gather on the PE: onehotT[k, p] = (chr[p % 64] == k+1),
            # rows = onehotT.T @ ce (exact: one 1.0 per column). chr arrives
            # broadcast to [32, 128] (host-side replication, same as
            # replicating ce across cores).
            nc.sync.dma_start(out=pre[:, :], in_=pre_h[:, :])
            nc.vector.tensor_scalar(
                out=oh[:, :],
                in0=pre[:, 0:128],
                scalar1=pre[:, 128:129],
                scalar2=None,
                op0=mybir.AluOpType.is_equal,
            )
            nc.tensor.matmul(
                rows_ps[:, :],
                oh[:, :],
                pre[:, 132 : 132 + DIM],
                start=True,
                stop=True,
            )
            nc.vector.tensor_copy(out=rep[:, 0:1, :], in_=rows_ps[:, :])

            # Replicate each partition's row W times along the free dim.
            w = 1
            while w < W:
                n = min(w, W - w)
                nc.vector.tensor_copy(out=rep[:, w : w + n, :], in_=rep[:, 0:n, :])
                w += n

            # Stream the output. The low partition half is pinned to the
            # sync HWDGE ring and the high half to the scalar ring so each
            # ring's source spans one SBUF port group; the openers enter
            # the SDMA rotation while the doubling chain is still running.
            _emit_ring(nc, nc.sync, out_h, rep, 0, BPC, 0, SYNC_BINS)
            _emit_ring(nc, nc.scalar, out_h, rep, BPC, 128, SYNC_BINS, NBIN)

    nc.compile()
    return nc


def make_in_maps(chr_full: np.ndarray, ce: np.ndarray):
    chr_f32 = chr_full.astype(np.float32)
    ce_pad = np.zeros((KPAD, DIM), np.float32)
    ce_pad[:N_CHR] = ce.astype(np.float32)
    maps = []
    for c in range(N_CORES):
        shard = chr_f32[c * BPC : (c + 1) * BPC]
        pre = np.zeros((KPAD, PRE_W), np.float32)
        pre[:, 0:128] = np.tile(shard, (KPAD, 2))  # chr broadcast
        pre[:, 128] = np.arange(1, KPAD + 1)  # iota
        pre[:, 132 : 132 + DIM] = ce_pad
        maps.append({"pre": np.ascontiguousarray(pre)})
    return maps


def kernel(tensor=None, chr=None, ce=None, **_unused):
    chr_np = np.asarray(chr)
    ce_np = np.asarray(ce)
    nc = build_nc()
    res = run_bass_kernel_spmd(
        nc, make_in_maps(chr_np, ce_np), core_ids=list(range(N_CORES))
    )
    out = np.concatenate([r["out"] for r in res.results], axis=0)
    return out.astype(np.float32)
